# revision 15
# baseline (speedup 1.0000x reference)
"""nn_GCNConv Trainium2 Bass kernel (8 NeuronCores, SPMD, no collectives).

Computation: out = segment_sum(features[src], dst, N) @ W + b
  features [10000,128] f32, edge_index [2,640000] i64, W [128,256], b [256]

Strategy (dense count-matrix SpMM, dst-node sharding -> no cross-core reduce):
  - Core c owns dst nodes [1250c, 1250c+1250).
  - The host re-encodes edge_index as a per-core COUNT matrix
    S_c[src, dst_local] = #edges(src -> 1250c+dst_local)   [10112 x 1250]
    (src padded 10000->10112 = 79*128 with zero rows; counts are small
    integers, exact in bf16/fp8e4m3). Pure integer bookkeeping - all
    feature arithmetic runs on device.
  - Device, per core, for each n-pass (dst column block 512/512/226):
      P1T[f, n] = sum_m feat[m, f] * S_c[m, n]   (79 K-tile accumulation
                  into one PSUM bank; lhsT = feat tile [128m,128f] bf16,
                  rhs = S tile [128m, pass_width])
      out[n, o] = sum_f P1T[f, n] * W[f, o] + b[o]   (projection + bias,
                  emitted so it overlaps the next pass's stream)
  - S streams from HBM in contiguous chunks (host pre-tiles to pass-major
    [128, sum_j 79*w_j] layout so every DMA is partition-contiguous);
    feat f32 is DMAed tiled once and cast to bf16 on device.
  - Host concatenates the 8 per-core [1250,256] outputs -> [10000,256].
"""

import sys

import numpy as np

_TRN_REPO = "/opt/trn_rl_repo"
if _TRN_REPO not in sys.path:
    sys.path.insert(0, _TRN_REPO)

import ml_dtypes  # noqa: E402

import concourse.bass as bass  # noqa: E402
import concourse.mybir as mybir  # noqa: E402
import concourse.tile as tile  # noqa: E402
from concourse import bacc, bass_utils  # noqa: E402

# ---------------------------------------------------------------------------
# Workaround: this walrus build rejects >1 sync-wait on a CTRL instruction
# ("Too many sync wait commands"). Tile's tail drain attaches a wait for every
# live sem lane to one InstDrain; chunk them across single-wait nops instead.
import re as _re  # noqa: E402

import bass_rust as _bass_rust  # noqa: E402


def _clock_ticks(vc):
    m = _re.search(r"\[([0-9, ]*)\]", repr(vc))
    return [int(x) for x in m.group(1).split(",")] if m.group(1).strip() else []


def _drain_and_barrier(self, tick_clock, wait_clock):
    ticks = _clock_ticks(tick_clock.global_clock)
    nz = [(i, t) for i, t in enumerate(ticks) if t > 0]
    for i, t in nz:
        vc = _bass_rust.VectorClock()
        vc.require_at_least(i, t)
        nop = self.nc.sync.nop(nofuse=True, hint="tail_wait")
        wait_clock.add_sem_waits(nop.ins, tile.ScopedClock({None: vc}))
    self.nc.sync.drain()  # waits already carried by the nops (SP FIFO order)
    self.nc.all_engine_barrier(sem_only=True)
    assert self.sems is not None
    popped = self.nc._tile_sem_poison_stack.pop()
    assert popped is self._sem_poison
    self.nc.clear_and_free_semaphores(list(self.sems.allocated().values()))
    self.nc.all_engine_barrier(sem_only=True)


tile.TileContext._drain_and_barrier = _drain_and_barrier
# ---------------------------------------------------------------------------

P = 128
C_IN = 128
C_OUT = 256
N_NODES = 10000
N_CORES = 8
NPC = N_NODES // N_CORES          # 1250 dst nodes per core
KT = 79                           # src K-tiles (10112 = 79*128)
K_PAD = KT * P                    # 10112
S_DTYPE = "fp8"                   # "bf16" | "fp8"  (DRAM storage of S)
PASS_W = [226, 512, 512]          # dst column blocks, one PSUM bank each
assert sum(PASS_W) == NPC
# t-tile chunking of each pass's S stream (pass 0 ramps up for a fast start)
S_CHUNKS = [
    [2, 3, 5, 8, 12, 16, 16, 17],
    [27, 26, 26],
    [27, 26, 26],
]
assert all(sum(cs) == KT for cs in S_CHUNKS)
F_CHUNKS = [2, 3, 5, 8, 12, 16, 16, 17]  # feat chunks, paired with pass-0 S
assert sum(F_CHUNKS) == KT
WARMUP_MM = 20                    # junk matmuls to pre-warm the PE clock


def _build_kernel(s_dtype: str):
    nc = bacc.Bacc("TRN2", num_swdge_queues=1)
    dt = mybir.dt
    s_dt = dt.bfloat16 if s_dtype == "bf16" else dt.float8e4

    feat_d = nc.dram_tensor("feat", [P, KT * C_IN], dt.bfloat16, kind="ExternalInput")
    st_d = nc.dram_tensor("st", [P, KT * NPC], s_dt, kind="ExternalInput")
    w_d = nc.dram_tensor("w", [C_IN, C_OUT], dt.float32, kind="ExternalInput")
    bb_d = nc.dram_tensor("bb", [P, C_OUT], dt.float32, kind="ExternalInput")
    out_d = nc.dram_tensor("out", [NPC, C_OUT], dt.float32, kind="ExternalOutput")

    with tile.TileContext(nc) as tc:
        with (
            tc.tile_pool(name="consts", bufs=1) as cpool,
            tc.tile_pool(name="fstage", bufs=2) as fpool,
            tc.tile_pool(name="schunk", bufs=8) as spool,
            tc.tile_pool(name="outs", bufs=2) as opool,
            tc.tile_pool(name="psa", bufs=1, space="PSUM") as psa,
            tc.tile_pool(name="pso", bufs=2, space="PSUM") as pso,
        ):

            # ---- PE warmup: junk matmuls into a scratch bank so the HAM
            # clock gate is at 8/8 before the first real matmul arrives ----
            wu_s = cpool.tile([P, P], dt.bfloat16)
            wu_p = psa.tile([P, 16], dt.float32)
            nc.vector.memset(wu_s[:], 0.0)
            wu_p2 = psa.tile([P, P], dt.float32)
            for _ in range(WARMUP_MM):
                nc.tensor.matmul(
                    wu_p2[:], lhsT=wu_s[:], rhs=wu_s[:], start=True, stop=True
                )

            # ---- constants (W cast + bias): deferred, only needed by the
            # first project() which runs during pass 1 ----
            w32_s = cpool.tile([C_IN, C_OUT], dt.float32)
            w_s = cpool.tile([C_IN, C_OUT], dt.bfloat16)
            bb_s = cpool.tile([P, C_OUT], dt.float32)
            consts_emitted = [False]

            def emit_consts():
                if not consts_emitted[0]:
                    nc.scalar.dma_start(out=w32_s[:], in_=w_d[:])
                    nc.scalar.dma_start(out=bb_s[:], in_=bb_d[:])
                    nc.vector.tensor_copy(w_s[:], w32_s[:])
                    consts_emitted[0] = True

            # ---- feat: direct bf16 DMA (host-tiled [p, t, c]), emitted
            # interleaved with pass-0 S chunks in consumption order ----
            feat_s = cpool.tile([P, KT, C_IN], dt.bfloat16)
            fview = feat_d[:].rearrange("p (t c) -> p t c", t=KT)
            f_off = [0]

            def emit_feat_chunk():
                k = len(f_off) - 1
                if k < len(F_CHUNKS):
                    a, fc = f_off[-1], F_CHUNKS[k]
                    nc.sync.dma_start(
                        out=feat_s[:, a : a + fc, :], in_=fview[:, a : a + fc, :]
                    )
                    f_off.append(a + fc)

            # ---- per-pass: stream S block, accumulate, project (overlapped) --
            p1t_ps = [
                psa.tile([P, 512], dt.float32, tag=f"p1t{j}", name=f"p1t{j}")
                for j in range(len(PASS_W))
            ]
            p1t_s = cpool.tile([P, NPC], dt.bfloat16)

            def project(j, n0, w):
                """Emit projection of finished pass j (PSUM -> out DRAM)."""
                nc.vector.tensor_copy(p1t_s[:, n0 : n0 + w], p1t_ps[j][:, :w])
                m0 = 0
                while m0 < w:
                    mm = min(P, w - m0)
                    out_p = pso.tile([P, C_OUT], dt.float32, tag="op", name="out_p")
                    nc.tensor.matmul(
                        out_p[:mm, :],
                        lhsT=p1t_s[:, n0 + m0 : n0 + m0 + mm],
                        rhs=w_s[:],
                        start=True,
                        stop=True,
                    )
                    out_s = opool.tile([P, C_OUT], dt.float32, tag="os", name="out_s")
                    nc.vector.tensor_add(
                        out_s[:mm, :], out_p[:mm, :], bb_s[:mm, :]
                    )
                    nc.scalar.dma_start(
                        out=out_d[n0 + m0 : n0 + m0 + mm, :], in_=out_s[:mm, :]
                    )
                    m0 += mm

            pending = None  # (j, n0, w) of the pass awaiting projection
            off = 0  # column offset into the pass-major st_d layout
            n0 = 0
            for j, w in enumerate(PASS_W):
                sview = st_d[:, off : off + KT * w].rearrange(
                    "p (t n) -> p t n", t=KT
                )
                ct_max = max(max(cs) for cs in S_CHUNKS)
                t0 = 0
                for ci, ct in enumerate(S_CHUNKS[j]):
                    if j == 0:
                        emit_feat_chunk()
                    s_s = spool.tile(
                        [P, ct_max, 512], s_dt, tag="s", name="s_s"
                    )
                    nc.sync.dma_start(
                        out=s_s[:, :ct, :w], in_=sview[:, t0 : t0 + ct, :]
                    )
                    if j == 1 and not consts_emitted[0]:
                        emit_consts()
                    for tl in range(ct):
                        t = t0 + tl
                        nc.tensor.matmul(
                            p1t_ps[j][:, :w],
                            lhsT=feat_s[:, t, :],
                            rhs=s_s[:, tl, :w],
                            start=(t == 0),
                            stop=(t == KT - 1),
                        )
                    t0 += ct
                    if ci == 0 and pending is not None:
                        # previous pass's projection overlaps this stream
                        project(*pending)
                        pending = None
                pending = (j, n0, w)
                off += KT * w
                n0 += w
            project(*pending)

    nc.compile()
    return nc


def _prep_inputs(features, edge_index, W, b, n_cores: int):
    """Host-side sharding: per-core count matrices + tiled feat layout.

    Pure data marshaling: edge_index -> exact integer count matrices,
    feat/W -> layout permutation + zero padding. No feature arithmetic.
    """
    s_np = ml_dtypes.bfloat16 if S_DTYPE == "bf16" else ml_dtypes.float8_e4m3

    src = np.asarray(edge_index[0], dtype=np.int64)
    dst = np.asarray(edge_index[1], dtype=np.int64)

    feat_np = np.zeros((K_PAD, C_IN), dtype=np.float32)
    feat_np[:N_NODES] = np.asarray(features, dtype=np.float32)
    feat_tiled = np.ascontiguousarray(
        feat_np.reshape(KT, P, C_IN)
        .transpose(1, 0, 2)
        .reshape(P, KT * C_IN)
        .astype(ml_dtypes.bfloat16)
    )
    w_np = np.ascontiguousarray(np.asarray(W, dtype=np.float32))
    bb_np = np.tile(np.asarray(b, dtype=np.float32)[None, :], (P, 1))

    order = np.argsort(dst, kind="stable")
    src_s = src[order]
    dst_s = dst[order]
    bounds = np.searchsorted(dst_s, np.arange(0, N_NODES + 1, NPC))

    in_maps = []
    for c in range(n_cores):
        lo, hi = bounds[c], bounds[c + 1]
        flat = src_s[lo:hi] * NPC + (dst_s[lo:hi] - c * NPC)
        cnt = np.bincount(flat, minlength=N_NODES * NPC)
        assert cnt.max() < 16, "edge multiplicity too large for exact fp8"
        cnt_pad = np.zeros((K_PAD, NPC), dtype=np.float32)
        cnt_pad[:N_NODES] = cnt.reshape(N_NODES, NPC)
        blocks = []
        n0 = 0
        for w in PASS_W:
            blocks.append(
                cnt_pad[:, n0 : n0 + w]
                .reshape(KT, P, w)
                .transpose(1, 0, 2)
                .reshape(P, KT * w)
            )
            n0 += w
        st = np.concatenate(blocks, axis=1).astype(s_np)
        in_maps.append(
            {
                "feat": feat_tiled,
                "st": np.ascontiguousarray(st),
                "w": w_np,
                "bb": bb_np,
            }
        )
    return in_maps


_KERNEL_CACHE: dict = {}


def _get_kernel():
    key = S_DTYPE
    if key not in _KERNEL_CACHE:
        _KERNEL_CACHE[key] = _build_kernel(S_DTYPE)
    return _KERNEL_CACHE[key]


def kernel(features, edge_index, W, b):
    features = np.asarray(features, dtype=np.float32)
    edge_index = np.asarray(edge_index)
    W = np.asarray(W, dtype=np.float32)
    b = np.asarray(b, dtype=np.float32)
    assert features.shape == (N_NODES, C_IN), features.shape
    assert W.shape == (C_IN, C_OUT) and b.shape == (C_OUT,)

    in_maps = _prep_inputs(features, edge_index, W, b, N_CORES)
    nc = _get_kernel()
    res = bass_utils.run_bass_kernel_spmd(nc, in_maps, core_ids=list(range(N_CORES)))
    out = np.concatenate([res.results[c]["out"] for c in range(N_CORES)], axis=0)
    return np.ascontiguousarray(out).astype(np.float32)


# revision 16
# speedup vs baseline: 1.1703x; 1.1703x over previous
"""nn_GCNConv Trainium2 Bass kernel (8 NeuronCores, SPMD, no collectives).

Computation: out = segment_sum(features[src], dst, N) @ W + b
  features [10000,128] f32, edge_index [2,640000] i64, W [128,256], b [256]

Strategy (dense count-matrix SpMM, dst-node sharding -> no cross-core reduce):
  - Core c owns dst nodes [1250c, 1250c+1250).
  - The host re-encodes edge_index as a per-core COUNT matrix
    S_c[src, dst_local] = #edges(src -> 1250c+dst_local)   [10112 x 1250]
    (src padded 10000->10112 = 79*128 with zero rows; counts are small
    integers, exact in bf16/fp8e4m3). Pure integer bookkeeping - all
    feature arithmetic runs on device.
  - Device, per core, for each n-pass (dst column block 512/512/226):
      P1T[f, n] = sum_m feat[m, f] * S_c[m, n]   (79 K-tile accumulation
                  into one PSUM bank; lhsT = feat tile [128m,128f] bf16,
                  rhs = S tile [128m, pass_width])
      out[n, o] = sum_f P1T[f, n] * W[f, o] + b[o]   (projection + bias,
                  emitted so it overlaps the next pass's stream)
  - S streams from HBM in contiguous chunks (host pre-tiles to pass-major
    [128, sum_j 79*w_j] layout so every DMA is partition-contiguous);
    feat f32 is DMAed tiled once and cast to bf16 on device.
  - Host concatenates the 8 per-core [1250,256] outputs -> [10000,256].
"""

import sys

import numpy as np

_TRN_REPO = "/opt/trn_rl_repo"
if _TRN_REPO not in sys.path:
    sys.path.insert(0, _TRN_REPO)

import ml_dtypes  # noqa: E402

import concourse.bass as bass  # noqa: E402
import concourse.mybir as mybir  # noqa: E402
import concourse.tile as tile  # noqa: E402
from concourse import bacc, bass_utils  # noqa: E402

# ---------------------------------------------------------------------------
# Workaround: this walrus build rejects >1 sync-wait on a CTRL instruction
# ("Too many sync wait commands"). Tile's tail drain attaches a wait for every
# live sem lane to one InstDrain; chunk them across single-wait nops instead.
import re as _re  # noqa: E402

import bass_rust as _bass_rust  # noqa: E402


def _clock_ticks(vc):
    m = _re.search(r"\[([0-9, ]*)\]", repr(vc))
    return [int(x) for x in m.group(1).split(",")] if m.group(1).strip() else []


def _drain_and_barrier(self, tick_clock, wait_clock):
    ticks = _clock_ticks(tick_clock.global_clock)
    nz = [(i, t) for i, t in enumerate(ticks) if t > 0]
    for i, t in nz:
        vc = _bass_rust.VectorClock()
        vc.require_at_least(i, t)
        nop = self.nc.sync.nop(nofuse=True, hint="tail_wait")
        wait_clock.add_sem_waits(nop.ins, tile.ScopedClock({None: vc}))
    self.nc.sync.drain()  # waits already carried by the nops (SP FIFO order)
    self.nc.all_engine_barrier(sem_only=True)
    assert self.sems is not None
    popped = self.nc._tile_sem_poison_stack.pop()
    assert popped is self._sem_poison
    self.nc.clear_and_free_semaphores(list(self.sems.allocated().values()))
    self.nc.all_engine_barrier(sem_only=True)


tile.TileContext._drain_and_barrier = _drain_and_barrier
# ---------------------------------------------------------------------------

P = 128
C_IN = 128
C_OUT = 256
N_NODES = 10000
N_CORES = 8
NPC = N_NODES // N_CORES          # 1250 dst nodes per core
KT = 79                           # src K-tiles (10112 = 79*128)
K_PAD = KT * P                    # 10112
S_DTYPE = "fp8"                   # "bf16" | "fp8"  (DRAM storage of S)
PASS_W = [512, 512, 226]          # dst column blocks, one PSUM bank each
assert sum(PASS_W) == NPC
# t-tile chunking of each pass's S stream (pass 0 ramps up for a fast start)
S_CHUNKS = [
    [2, 3, 5, 8, 12, 16, 16, 17],
    [16, 16, 16, 16, 15],
    [16, 16, 16, 16, 15],
]
assert all(sum(cs) == KT for cs in S_CHUNKS)
F_CHUNKS = [2, 3, 5, 8, 12, 16, 16, 17]  # feat chunks, paired with pass-0 S
assert sum(F_CHUNKS) == KT
WARMUP_MM = 20                    # junk matmuls to pre-warm the PE clock


def _build_kernel(s_dtype: str):
    nc = bacc.Bacc("TRN2", num_swdge_queues=1)
    dt = mybir.dt
    s_dt = dt.bfloat16 if s_dtype == "bf16" else dt.float8e4

    feat_d = nc.dram_tensor("feat", [P, KT * C_IN], dt.bfloat16, kind="ExternalInput")
    st_d = nc.dram_tensor("st", [P, KT * NPC], s_dt, kind="ExternalInput")
    w_d = nc.dram_tensor("w", [C_IN, C_OUT], dt.float32, kind="ExternalInput")
    bb_d = nc.dram_tensor("bb", [P, C_OUT], dt.float32, kind="ExternalInput")
    out_d = nc.dram_tensor("out", [NPC, C_OUT], dt.float32, kind="ExternalOutput")

    with tile.TileContext(nc) as tc:
        with (
            tc.tile_pool(name="consts", bufs=1) as cpool,
            tc.tile_pool(name="fstage", bufs=2) as fpool,
            tc.tile_pool(name="schunk", bufs=8) as spool,
            tc.tile_pool(name="outs", bufs=2) as opool,
            tc.tile_pool(name="psa", bufs=1, space="PSUM") as psa,
            tc.tile_pool(name="pso", bufs=2, space="PSUM") as pso,
        ):

            # ---- PE warmup: junk matmuls into a scratch bank so the HAM
            # clock gate is at 8/8 before the first real matmul arrives ----
            wu_s = cpool.tile([P, P], dt.bfloat16)
            wu_p = psa.tile([P, 16], dt.float32)
            nc.vector.memset(wu_s[:], 0.0)
            wu_p2 = psa.tile([P, P], dt.float32)
            for _ in range(WARMUP_MM):
                nc.tensor.matmul(
                    wu_p2[:], lhsT=wu_s[:], rhs=wu_s[:], start=True, stop=True
                )

            # ---- constants (W cast + bias): deferred, only needed by the
            # first project() which runs during pass 1 ----
            w32_s = cpool.tile([C_IN, C_OUT], dt.float32)
            w_s = cpool.tile([C_IN, C_OUT], dt.bfloat16)
            bb_s = cpool.tile([P, C_OUT], dt.float32)
            consts_emitted = [False]

            def emit_consts():
                if not consts_emitted[0]:
                    nc.scalar.dma_start(out=w32_s[:], in_=w_d[:])
                    nc.scalar.dma_start(out=bb_s[:], in_=bb_d[:])
                    nc.vector.tensor_copy(w_s[:], w32_s[:])
                    consts_emitted[0] = True

            # ---- feat: direct bf16 DMA (host-tiled [p, t, c]), emitted
            # interleaved with pass-0 S chunks in consumption order ----
            feat_s = cpool.tile([P, KT, C_IN], dt.bfloat16)
            fview = feat_d[:].rearrange("p (t c) -> p t c", t=KT)
            f_off = [0]

            def emit_feat_chunk():
                k = len(f_off) - 1
                if k < len(F_CHUNKS):
                    a, fc = f_off[-1], F_CHUNKS[k]
                    nc.sync.dma_start(
                        out=feat_s[:, a : a + fc, :], in_=fview[:, a : a + fc, :]
                    )
                    f_off.append(a + fc)

            # ---- per-pass: stream S block, accumulate, project (overlapped) --
            p1t_ps = [
                psa.tile([P, 512], dt.float32, tag=f"p1t{j}", name=f"p1t{j}")
                for j in range(len(PASS_W))
            ]
            p1t_s = cpool.tile([P, NPC], dt.bfloat16)

            def project(j, n0, w):
                """Emit projection of finished pass j (PSUM -> out DRAM)."""
                nc.vector.tensor_copy(p1t_s[:, n0 : n0 + w], p1t_ps[j][:, :w])
                m0 = 0
                while m0 < w:
                    mm = min(P, w - m0)
                    out_p = pso.tile([P, C_OUT], dt.float32, tag="op", name="out_p")
                    nc.tensor.matmul(
                        out_p[:mm, :],
                        lhsT=p1t_s[:, n0 + m0 : n0 + m0 + mm],
                        rhs=w_s[:],
                        start=True,
                        stop=True,
                    )
                    out_s = opool.tile([P, C_OUT], dt.float32, tag="os", name="out_s")
                    nc.vector.tensor_add(
                        out_s[:mm, :], out_p[:mm, :], bb_s[:mm, :]
                    )
                    nc.scalar.dma_start(
                        out=out_d[n0 + m0 : n0 + m0 + mm, :], in_=out_s[:mm, :]
                    )
                    m0 += mm

            pending = None  # (j, n0, w) of the pass awaiting projection
            off = 0  # column offset into the pass-major st_d layout
            n0 = 0
            for j, w in enumerate(PASS_W):
                sview = st_d[:, off : off + KT * w].rearrange(
                    "p (t n) -> p t n", t=KT
                )
                ct_max = max(max(cs) for cs in S_CHUNKS)
                t0 = 0
                for ci, ct in enumerate(S_CHUNKS[j]):
                    if j == 0:
                        emit_feat_chunk()
                    s_s = spool.tile(
                        [P, ct_max, 512], s_dt, tag="s", name="s_s"
                    )
                    nc.sync.dma_start(
                        out=s_s[:, :ct, :w], in_=sview[:, t0 : t0 + ct, :]
                    )
                    if j == 1 and not consts_emitted[0]:
                        emit_consts()
                    for tl in range(ct):
                        t = t0 + tl
                        nc.tensor.matmul(
                            p1t_ps[j][:, :w],
                            lhsT=feat_s[:, t, :],
                            rhs=s_s[:, tl, :w],
                            start=(t == 0),
                            stop=(t == KT - 1),
                        )
                    t0 += ct
                    if ci == 0 and pending is not None:
                        # previous pass's projection overlaps this stream
                        project(*pending)
                        pending = None
                pending = (j, n0, w)
                off += KT * w
                n0 += w
            project(*pending)

    nc.compile()
    return nc


def _prep_inputs(features, edge_index, W, b, n_cores: int):
    """Host-side sharding: per-core count matrices + tiled feat layout.

    Pure data marshaling: edge_index -> exact integer count matrices,
    feat/W -> layout permutation + zero padding. No feature arithmetic.
    """
    s_np = ml_dtypes.bfloat16 if S_DTYPE == "bf16" else ml_dtypes.float8_e4m3

    src = np.asarray(edge_index[0], dtype=np.int64)
    dst = np.asarray(edge_index[1], dtype=np.int64)

    feat_np = np.zeros((K_PAD, C_IN), dtype=np.float32)
    feat_np[:N_NODES] = np.asarray(features, dtype=np.float32)
    feat_tiled = np.ascontiguousarray(
        feat_np.reshape(KT, P, C_IN)
        .transpose(1, 0, 2)
        .reshape(P, KT * C_IN)
        .astype(ml_dtypes.bfloat16)
    )
    w_np = np.ascontiguousarray(np.asarray(W, dtype=np.float32))
    bb_np = np.tile(np.asarray(b, dtype=np.float32)[None, :], (P, 1))

    order = np.argsort(dst, kind="stable")
    src_s = src[order]
    dst_s = dst[order]
    bounds = np.searchsorted(dst_s, np.arange(0, N_NODES + 1, NPC))

    in_maps = []
    for c in range(n_cores):
        lo, hi = bounds[c], bounds[c + 1]
        flat = src_s[lo:hi] * NPC + (dst_s[lo:hi] - c * NPC)
        cnt = np.bincount(flat, minlength=N_NODES * NPC)
        assert cnt.max() < 16, "edge multiplicity too large for exact fp8"
        cnt_pad = np.zeros((K_PAD, NPC), dtype=np.float32)
        cnt_pad[:N_NODES] = cnt.reshape(N_NODES, NPC)
        blocks = []
        n0 = 0
        for w in PASS_W:
            blocks.append(
                cnt_pad[:, n0 : n0 + w]
                .reshape(KT, P, w)
                .transpose(1, 0, 2)
                .reshape(P, KT * w)
            )
            n0 += w
        st = np.concatenate(blocks, axis=1).astype(s_np)
        in_maps.append(
            {
                "feat": feat_tiled,
                "st": np.ascontiguousarray(st),
                "w": w_np,
                "bb": bb_np,
            }
        )
    return in_maps


_KERNEL_CACHE: dict = {}


def _get_kernel():
    key = S_DTYPE
    if key not in _KERNEL_CACHE:
        _KERNEL_CACHE[key] = _build_kernel(S_DTYPE)
    return _KERNEL_CACHE[key]


def kernel(features, edge_index, W, b):
    features = np.asarray(features, dtype=np.float32)
    edge_index = np.asarray(edge_index)
    W = np.asarray(W, dtype=np.float32)
    b = np.asarray(b, dtype=np.float32)
    assert features.shape == (N_NODES, C_IN), features.shape
    assert W.shape == (C_IN, C_OUT) and b.shape == (C_OUT,)

    in_maps = _prep_inputs(features, edge_index, W, b, N_CORES)
    nc = _get_kernel()
    res = bass_utils.run_bass_kernel_spmd(nc, in_maps, core_ids=list(range(N_CORES)))
    out = np.concatenate([res.results[c]["out"] for c in range(N_CORES)], axis=0)
    return np.ascontiguousarray(out).astype(np.float32)
